# revision 23
# baseline (speedup 1.0000x reference)
"""Trainium2 Bass kernel for nn_ModelR_37022618091886.

Model: y = MLP(x) with 5 layers (leaky-relu 0.01 between), then per-example
triangular scatter of the 2080 outputs into an upper-triangular 64x64 matrix
(diagonal entries abs'ed), output shape (64, 64, 8192).

Strategy:
  - Data-parallel over batch across 8 cores (1024 examples/core), weights
    replicated.
  - Activations are kept feature-major on chip: h^T [features, batch], so the
    contraction dim of every matmul is already on SBUF partitions and the
    final layer directly produces y^T [2080, batch] = (almost) the output.
  - W3's columns are permuted on the host so that the final layer's output
    rows land in output-row order: row block i holds (i,j) for j=i..63
    ascending.  The "scatter" then degenerates to ~80 contiguous-run DMAs.
  - The diagonal abs is y = max(y, c*y) with c = -1 on diagonal rows else +1
    (per-partition scalar), fused after the bias add.
  - Strict-lower-triangle rows of the output are never written; the runtime
    pre-zeroes/donates zeroed output buffers (see run_bass_via_pjrt), so they
    read back as exact zeros.
  - Matmuls run in bf16 (same 1 row/cycle PE rate as fp32r, half the DMA
    bytes, and the weight-load path pipelines into the PE background buffer
    instead of the f32r 4-byte self-load).  Normalized error ~6e-3.
  - Weights are pre-tiled on the host to [mt*128, kt*128] so each m-block
    weight load is a single contiguous-per-partition DMA, issued from the
    otherwise-idle gpsimd queue with a 16-deep buffer pool (deep prefetch;
    kicks from the sync/scalar queues serialize behind event-sems and
    activations and stall the PE under 8-core load).
  - Output is stored bf16 and upcast on the host: halves the L5 store
    traffic, which measurably contends with weight reads under 8-core load.
    Store kicks also issue from gpsimd (the sync queue's event-semaphore
    forwarding congests during the store-heavy final layer).
  - Post-schedule passes: drop InstLdweights reloads for back-to-back
    matmuls on the same weights (the ps0/ps1 chunk pairs), and thin the
    per-matmul PE counting-semaphore incs to group keepers (renumbering all
    wait thresholds to match).
"""

import os

import numpy as np

CPV = 64
L = CPV * (CPV + 1) // 2  # 2080
LT = 17  # number of 128-row tiles covering L (padded)
LP = LT * 128  # 2176
D_IN = 1024
H = 2048
B = 8192
N_CORES = 8
BC = B // N_CORES  # 1024 batch per core
NCH = 2  # moving-dim chunks of 512 (fp32 PSUM bank limit)
NSZ = BC // NCH

_DTYPE = os.environ.get("KERNEL_DTYPE", "bf16")  # "bf16" | "f32r" | "f32"
_REPS = int(os.environ.get("KERNEL_REPS", "1"))  # timing aid: unroll kernel R times
_OPT_DEDUP = os.environ.get("KERNEL_NO_DEDUP", "") == ""
# Strip per-matmul PE-counting-semaphore incs down to group keepers and
# renumber every wait threshold to count keepers only (walrus requires
# UpdateValue == 1, so incs cannot be merged by value).
_OPT_RENUMBER = os.environ.get("KERNEL_NO_RENUMBER", "") == ""

_compiled_nc = None


def _post_optimize(nc, dedup=True, coalesce=True):
    """Post-schedule PE-stream optimizations (run before nc.compile()).

    1. LDWEIGHTS dedup: bass emits one InstLdweights per matmul; consecutive
       matmuls that reuse the same stationary weights (our ps0/ps1 batch-chunk
       pairs) reload identical weights.  The PE array keeps weights across
       matmuls, so the repeat load is pure overhead (~53-107ns serial, since
       full-K LDWEIGHTS can never overlap in-flight matmuls).
    2. Semaphore-inc coalescing: every matmul carries a +1 update on the PE
       counting semaphore (~26ns serialized register write each).  Only counts
       at accumulation-group boundaries (stop matmuls) and at wait-carriers
       are consumed promptly by other engines; intermediate incs are deferred
       onto the next keeper, preserving the total count and all "sem >= N"
       thresholds at every consumption point that matters.
    """
    import concourse.mybir as mybir

    PE = mybir.EngineType.PE
    for fn in nc.m.functions:
        for blk in fn.blocks:
            insts = blk.instructions
            if dedup:
                prev_sig = None
                keep = []
                for i in insts:
                    tn = type(i).__name__
                    if getattr(i, "engine", None) == PE:
                        if tn == "InstLdweights":
                            si = i.sync_info
                            clean = not (si and (list(si.on_wait) or list(si.on_update)))
                            sig = (
                                str(i.ins[0]),
                                str(i.perf_mode),
                                str(i.is_transpose),
                                str(i.tile_position),
                            )
                            if clean and sig == prev_sig:
                                continue  # identical reload: drop
                            prev_sig = sig
                        elif tn != "InstMatmult":
                            prev_sig = None  # unknown PE effect: invalidate
                    keep.append(i)
                if len(keep) != len(insts):
                    blk.instructions = keep
                    insts = keep
    if coalesce:
        _renumber_pe_sem(nc)
    return nc


def _renumber_pe_sem(nc):
    """Thin out the PE counting-semaphore increments.

    Every matmul carries a +1 on the engine counting sem (PE_<core>); each is
    a serialized EVT_SEM register write (~26ns).  Consumers only ever need
    counts at accumulation-group ends (stop matmuls) or just before a PE
    instruction that itself waits (else the waiter could deadlock a consumer
    chain).  Keep incs only there and rewrite every literal `sem >= V` wait
    against the new keeper-counting: V -> index of first keeper whose
    original cumulative count >= V.
    """
    import concourse.mybir as mybir

    PE = mybir.EngineType.PE

    # ---- gather the global instruction order and the PE_49-inc events ----
    sem_name = None
    events = []  # (inst, si, cum_old_after, strippable)
    cum = 0
    pe_blocks = set()
    all_insts = []
    for fn in nc.m.functions:
        for bidx, blk in enumerate(fn.blocks):
            for i in blk.instructions:
                all_insts.append(i)
                si = i.sync_info
                if not si:
                    continue
                for u in si.on_update:
                    if u.ant_name and u.ant_name.startswith("PE_"):
                        if sem_name is None:
                            sem_name = u.ant_name
                        assert u.ant_name == sem_name and u.update_mode == "sem-inc"
                        assert u.update_value == 1 and u.update_reg is None
                        cum += 1
                        strippable = (
                            type(i).__name__ == "InstMatmult"
                            and len(list(si.on_update)) == 1
                            and not i.stop_tensor_calc
                        )
                        events.append([i, si, cum, strippable])
                        pe_blocks.add(bidx)
    if not events:
        return
    assert len(pe_blocks) == 1, f"PE incs span blocks {pe_blocks}; need linear order"

    # ---- keepers: stop/non-strippable MMs, last event before a PE waiter,
    #      and the final event ----
    # walk the PE stream again to find "last inc-event before each PE wait"
    ev_idx = {id(e[0]): k for k, e in enumerate(events)}
    keeper = [not e[3] for e in events]
    last_seen = -1
    for i in all_insts:
        if getattr(i, "engine", None) != PE:
            continue
        k = ev_idx.get(id(i))
        si = i.sync_info
        if si and list(si.on_wait) and last_seen >= 0:
            keeper[last_seen] = True
        if k is not None:
            last_seen = k
    keeper[-1] = True

    # ---- old-cumulative -> new-cumulative mapping ----
    # new count after keeper j (1-based index among keepers)
    keeper_cum_old = []  # ascending old cum at each keeper
    for k, e in enumerate(events):
        if keeper[k]:
            keeper_cum_old.append(e[2])
    import bisect

    def remap(v_old):
        j = bisect.bisect_left(keeper_cum_old, v_old)
        assert j < len(keeper_cum_old), (v_old, keeper_cum_old[-1])
        return j + 1

    # ---- apply: strip non-keeper incs, rewrite waits ----
    for k, e in enumerate(events):
        if not keeper[k]:
            e[1].on_update = []
    n_rewritten = 0
    for i in all_insts:
        si = i.sync_info
        if not si:
            continue
        ws = list(si.on_wait)
        changed = False
        for w in ws:
            if w.ant_name == sem_name:
                assert w.wait_mode == "sem-ge-imm" and w.wait_reg is None
                w.wait_value = remap(w.wait_value)
                changed = True
                n_rewritten += 1
        if changed:
            si.on_wait = ws


def _np_mmdt():
    if _DTYPE == "bf16":
        import ml_dtypes

        return ml_dtypes.bfloat16
    return np.float32


def _offsets():
    off = np.zeros(CPV + 1, dtype=np.int64)
    for i in range(CPV):
        off[i + 1] = off[i] + (CPV - i)
    return off


def _perm_and_coefs():
    """Column permutation for W3 + diag coefficient vector.

    New output order m: for i in 0..63, for j in i..63 -> m = off[i] + (j-i).
    Torch/ref order k: for i, for idx_y: col j = 63-idx_y -> k = off[i] + (63-j).
    """
    off = _offsets()
    perm = np.empty(L, dtype=np.int64)
    for i in range(CPV):
        n = CPV - i
        perm[off[i] : off[i] + n] = off[i] + np.arange(n)[::-1]
    coef = np.ones(LP, dtype=np.float32)
    coef[off[:CPV]] = -1.0  # diagonal (i,i) sits at the start of block i
    return perm, coef


def _out_runs():
    """Maximal runs of consecutive final-layer rows that map to consecutive
    output rows: list of (tile, p0, length, out_row0)."""
    off = _offsets()
    runs = []
    for t in range(LT):
        m0t, m1t = t * 128, min((t + 1) * 128, L)
        m = m0t
        while m < m1t:
            i = int(np.searchsorted(off, m, side="right") - 1)
            end = int(min(m1t, off[i] + (CPV - i)))
            runs.append((t, m - m0t, end - m, 65 * i + (m - off[i])))
            m = end
    assert sum(r[2] for r in runs) == L
    return runs


def _build(wp_bufs=None, ps_bufs=8, yp_bufs=2, reps=None):
    if wp_bufs is None:
        wp_bufs = int(os.environ.get("KERNEL_WP_BUFS", "16"))
    n_queues = int(os.environ.get("KERNEL_W_QUEUES", "2"))
    import concourse.bacc as bacc
    import concourse.mybir as mybir
    import concourse.tile as tile

    F32 = mybir.dt.float32
    MMDT = {
        "bf16": mybir.dt.bfloat16,
        "f32r": mybir.dt.float32r,
        "f32": mybir.dt.float32,
    }[_DTYPE]
    ACT = mybir.ActivationFunctionType

    nc = bacc.Bacc("TRN2", target_bir_lowering=False, debug=False, num_devices=N_CORES)

    # Weights arrive host-pre-tiled: [mt*128, kt*128], row m*128+p holding
    # W[k*128 + (col k-idx), m*128 + (col c)] ... i.e. wt[p, :] for block m is
    # exactly rows m*128..m*128+127.
    xt = nc.dram_tensor("xt", [D_IN, BC], MMDT, kind="ExternalInput")
    w1 = nc.dram_tensor("w1", [H, D_IN], MMDT, kind="ExternalInput")
    w2 = nc.dram_tensor("w2", [H, H], MMDT, kind="ExternalInput")
    w21 = nc.dram_tensor("w21", [H, H], MMDT, kind="ExternalInput")
    w22 = nc.dram_tensor("w22", [H, H], MMDT, kind="ExternalInput")
    w3 = nc.dram_tensor("w3", [LP, H], MMDT, kind="ExternalInput")
    b1t = nc.dram_tensor("b1t", [128, H // 128], F32, kind="ExternalInput")
    b2t = nc.dram_tensor("b2t", [128, H // 128], F32, kind="ExternalInput")
    b21t = nc.dram_tensor("b21t", [128, H // 128], F32, kind="ExternalInput")
    b22t = nc.dram_tensor("b22t", [128, H // 128], F32, kind="ExternalInput")
    b3t = nc.dram_tensor("b3t", [128, LT], F32, kind="ExternalInput")
    c3t = nc.dram_tensor("c3t", [128, LT], F32, kind="ExternalInput")
    out_bf16 = os.environ.get("KERNEL_OUT_BF16", "1") != ""
    ODT = mybir.dt.bfloat16 if out_bf16 else F32
    out = nc.dram_tensor("out", [CPV * CPV, BC], ODT, kind="ExternalOutput")

    runs = _out_runs()

    with tile.TileContext(nc) as tc:
        with (
            tc.tile_pool(name="acts", bufs=1) as acts,
            tc.tile_pool(name="wp", bufs=wp_bufs) as wp,
            tc.tile_pool(name="cst", bufs=1) as cst,
            tc.tile_pool(name="yp", bufs=yp_bufs) as yp,
            tc.tile_pool(name="ycp", bufs=1) as ycp,
            tc.tile_pool(name="ps", bufs=ps_bufs, space="PSUM") as ps,
        ):
            # --- constants (biases / diag coefs) ---
            bias_tiles = {}
            for name, dram, nt in (
                ("b1", b1t, 16),
                ("b2", b2t, 16),
                ("b21", b21t, 16),
                ("b22", b22t, 16),
                ("b3", b3t, LT),
                ("c3", c3t, LT),
            ):
                t = cst.tile([128, nt], F32, tag=name, name=name)
                nc.sync.dma_start(t[:], dram[:, :])
                bias_tiles[name] = t

            for _rep in range(_REPS if reps is None else reps):
              # --- x^T load: 8 k-tiles, alternating DMA queues ---
              x_tiles = []
              for k in range(D_IN // 128):
                t = acts.tile([128, BC], MMDT, tag=f"x{k}", name=f"x{k}")
                eng = nc.sync if k % 2 == 0 else nc.scalar
                eng.dma_start(t[:], xt[k * 128 : (k + 1) * 128, :])
                x_tiles.append(t)

              out_eng = {"s": nc.sync, "a": nc.scalar, "g": nc.gpsimd}[
                  os.environ.get("KERNEL_OUT_ENG", "g")
              ]
              w_eng_spec = os.environ.get("KERNEL_W_ENG", "g")
              w_engines = [
                  {"s": nc.sync, "a": nc.scalar, "g": nc.gpsimd}[c] for c in w_eng_spec
              ][:n_queues]
              fake_w = os.environ.get("KERNEL_FAKE_W", "") != ""
              w_half = int(os.environ.get("KERNEL_W_HALF", "1"))  # m-blocks per DMA
              fake_tiles = {}
              half_tiles = {}

              def get_weight_block(lname, w_dram, kt, m, mt):
                  """Weight m-block view; loads arrive as half-layer mega-DMAs
                  (w_half m-blocks per DMA) into a rotating pool."""
                  if fake_w:
                      # timing probe: one static weight tile per layer, no
                      # per-m streaming (numerics are wrong on purpose)
                      if lname not in fake_tiles:
                          wt = cst.tile(
                              [128, kt * 128], MMDT, tag=f"fw_{lname}", name=f"fw_{lname}"
                          )
                          nc.sync.dma_start(wt[:], w_dram[:128, :])
                          fake_tiles[lname] = wt
                      return fake_tiles[lname]
                  m0 = (m // w_half) * w_half
                  key = (lname, m0)
                  if key not in half_tiles:
                      nm = min(w_half, mt - m0)
                      t = wp.tile(
                          [128, nm * kt * 128], MMDT, tag="w", name=f"lw_{lname}_{m0}"
                      )
                      eng = w_engines[(m0 // w_half) % len(w_engines)]
                      eng.dma_start(
                          t[:].rearrange("p (m c) -> p m c", m=nm),
                          w_dram[m0 * 128 : (m0 + nm) * 128, :].rearrange(
                              "(m p) c -> p m c", p=128
                          ),
                      )
                      half_tiles[key] = t
                  t = half_tiles[(lname, m0)]
                  off = (m - m0) * kt * 128
                  return t[:, off : off + kt * 128]

              def layer(lname, w_dram, kt, mt, h_in, btile, out_tag_prefix):
                  """One hidden layer: h_out[m] = lrelu(W[:,m]^T @ h_in + b[m])."""
                  h_out = []
                  for m in range(mt):
                      wt = get_weight_block(lname, w_dram, kt, m, mt)
                      ps0 = ps.tile([128, NSZ], F32, tag="ps", name=f"ps0_{lname}_{m}")
                      ps1 = ps.tile([128, NSZ], F32, tag="ps", name=f"ps1_{lname}_{m}")
                      for k in range(kt):
                          lhsT = wt[:, k * 128 : (k + 1) * 128]
                          nc.tensor.matmul(
                              ps0[:],
                              lhsT,
                              h_in[k][:, :NSZ],
                              start=(k == 0),
                              stop=(k == kt - 1),
                          )
                          nc.tensor.matmul(
                              ps1[:],
                              lhsT,
                              h_in[k][:, NSZ:],
                              start=(k == 0),
                              stop=(k == kt - 1),
                          )
                      ht = acts.tile(
                          [128, BC],
                          MMDT,
                          tag=f"{out_tag_prefix}{m}",
                          name=f"h_{lname}_{m}",
                      )
                      nc.scalar.activation(
                          ht[:, :NSZ],
                          ps0[:],
                          ACT.Lrelu,
                          bias=btile[:, m : m + 1],
                          scale=1.0,
                          alpha=0.01,
                      )
                      nc.scalar.activation(
                          ht[:, NSZ:],
                          ps1[:],
                          ACT.Lrelu,
                          bias=btile[:, m : m + 1],
                          scale=1.0,
                          alpha=0.01,
                      )
                      h_out.append(ht)
                  return h_out

              h1 = layer("l1", w1, D_IN // 128, H // 128, x_tiles, bias_tiles["b1"], "a")
              h2 = layer("l2", w2, H // 128, H // 128, h1, bias_tiles["b2"], "b")
              h3 = layer("l3", w21, H // 128, H // 128, h2, bias_tiles["b21"], "a")
              h4 = layer("l4", w22, H // 128, H // 128, h3, bias_tiles["b22"], "b")

              # --- final layer + scatter ---
              runs_by_tile = {}
              for r in runs:
                  runs_by_tile.setdefault(r[0], []).append(r)
              kt = H // 128
              for m in range(LT):
                  wt = get_weight_block("l5", w3, kt, m, LT)
                  ps0 = ps.tile([128, NSZ], F32, tag="ps", name=f"ps0_l5_{m}")
                  ps1 = ps.tile([128, NSZ], F32, tag="ps", name=f"ps1_l5_{m}")
                  for k in range(kt):
                      lhsT = wt[:, k * 128 : (k + 1) * 128]
                      nc.tensor.matmul(
                          ps0[:], lhsT, h4[k][:, :NSZ], start=(k == 0), stop=(k == kt - 1)
                      )
                      nc.tensor.matmul(
                          ps1[:], lhsT, h4[k][:, NSZ:], start=(k == 0), stop=(k == kt - 1)
                      )
                  y = yp.tile([128, BC], F32, tag="y", name=f"y_{m}")
                  nc.scalar.activation(
                      y[:, :NSZ],
                      ps0[:],
                      ACT.Identity,
                      bias=bias_tiles["b3"][:, m : m + 1],
                  )
                  nc.scalar.activation(
                      y[:, NSZ:],
                      ps1[:],
                      ACT.Identity,
                      bias=bias_tiles["b3"][:, m : m + 1],
                  )
                  yo = y
                  if out_bf16:
                      yo = yp.tile([128, BC], ODT, tag="yb", name=f"yb_{m}")
                  # yo = max(y * c, y): diag rows (c=-1) -> |y|, others -> y
                  nc.vector.scalar_tensor_tensor(
                      yo[:],
                      y[:],
                      bias_tiles["c3"][:, m : m + 1],
                      y[:],
                      mybir.AluOpType.mult,
                      mybir.AluOpType.max,
                  )
                  for _, p0, ln, r0 in runs_by_tile.get(m, []):
                      out_eng.dma_start(out[r0 : r0 + ln, :], yo[p0 : p0 + ln, :])

    _post_optimize(nc, dedup=_OPT_DEDUP, coalesce=_OPT_RENUMBER)
    nc.compile()
    return nc


def _get_nc():
    global _compiled_nc
    if _compiled_nc is None:
        _compiled_nc = _build()
    return _compiled_nc


def _tile_weight(W):
    """[K, M] -> [MT*128, KT*128] host pre-tiling: row m*128+p, col k*128+c
    holds W[k*128+p, m*128+c], so weight block m is rows m*128..m*128+127."""
    K, M = W.shape
    kt, mt = K // 128, M // 128
    return np.ascontiguousarray(
        W.reshape(kt, 128, mt, 128).transpose(2, 1, 0, 3).reshape(mt * 128, kt * 128)
    )


def prepare_in_maps(x, W1, b1, W2, b2, W21, b21, W22, b22, W3, b3):
    mmdt = _np_mmdt()
    x = np.asarray(x, dtype=np.float32)
    perm, coef = _perm_and_coefs()
    w3p = np.zeros((H, LP), dtype=np.float32)
    w3p[:, :L] = np.asarray(W3, np.float32)[:, perm]
    b3p = np.zeros(LP, dtype=np.float32)
    b3p[:L] = np.asarray(b3, np.float32)[perm]

    def tile_bias(b, nt):
        return np.ascontiguousarray(np.asarray(b, np.float32).reshape(nt, 128).T)

    common = {
        "w1": _tile_weight(np.asarray(W1, np.float32)).astype(mmdt),
        "w2": _tile_weight(np.asarray(W2, np.float32)).astype(mmdt),
        "w21": _tile_weight(np.asarray(W21, np.float32)).astype(mmdt),
        "w22": _tile_weight(np.asarray(W22, np.float32)).astype(mmdt),
        "w3": _tile_weight(w3p).astype(mmdt),
        "b1t": tile_bias(b1, 16),
        "b2t": tile_bias(b2, 16),
        "b21t": tile_bias(b21, 16),
        "b22t": tile_bias(b22, 16),
        "b3t": tile_bias(b3p, LT),
        "c3t": tile_bias(coef, LT),
    }
    xT = np.ascontiguousarray(x.T).astype(mmdt)  # [D_IN, B]
    return [
        {**common, "xt": np.ascontiguousarray(xT[:, c * BC : (c + 1) * BC])}
        for c in range(N_CORES)
    ]


def kernel(
    x, W1, b1, W2, b2, W21, b21, W22, b22, W3, b3
):  # noqa: N803 - match reference names
    nc = _get_nc()
    in_maps = prepare_in_maps(x, W1, b1, W2, b2, W21, b21, W22, b22, W3, b3)

    from concourse.bass_utils import run_bass_kernel_spmd

    res = run_bass_kernel_spmd(nc, in_maps, core_ids=list(range(N_CORES)))
    return np.concatenate(
        [
            np.asarray(res.results[c]["out"], dtype=np.float32).reshape(CPV, CPV, BC)
            for c in range(N_CORES)
        ],
        axis=2,
    )
